# revision 17
# baseline (speedup 1.0000x reference)
"""Trainium2 Bass kernel for nn_DiffusionModel (theta_post_prob).

Math (per batch b, with runtime scalars a = alphas-gather, ca = cumalphas-gather):
    p     = a*xt + k1                 k1 = (1-a)/C
    M     = ca*I + u*ones             u  = (1-ca)/C   (C x C, symmetric, stochastic)
    denom = M^T p
    g     = theta_x0 / denom
    out   = p * (M g)

Key identity: xt is class-normalized (sum_c xt = 1), so sum_c p = a + C*k1 = 1
and therefore denom = ca*p + u = (ca*a)*xt + (ca*k1 + u) — a pure elementwise
affine of xt. The first class-reduction needs NO matmul; only M g does.

Kernel layout: batch b -> core b (pure data parallel, 8 cores). Per core the
(C=32, HW=65536) slab is processed as [128, N] tiles where the 128 partitions
pack G=4 independent spatial blocks x 32 classes. The M g reduction is a PE
matmul against the block-diagonal 128x128 matrix kron(M, I4) built on host
(partition p = class*4 + block, so DRAM rows sit at a uniform 64 KiB stride).

All HBM I/O is bf16 (host casts inputs down and the output back up), halving
DMA traffic vs fp32 — max rel err ~1.3e-2 vs the 2e-2 gate on the reference's
deterministic inputs. Matmul operands (mb, g) are bf16: 1 cyc/row even at
cold PE clock and cheap LDWEIGHTS. p/dn/rden stay f32 in SBUF. Elementwise
ops run at full DMA-tile width (2048) except the o-mult, which matches the
1024-col PSUM tiles. All loads ride the sync HWDGE ring (pure prefetch,
never blocked by compute); stores issue from gpsimd (SWDGE) deferred by one
iteration, so the store's o tile is already complete when the issue executes
and the engine FIFO never stalls on it.

Engine budget per 2048-col iteration (8 iterations/core):
    scalar: p-act + dn-act          (~4.6 us)
    vector: rden recip + 2 o-mults  (~4.7 us)
    gpsimd: g-mul + store issue     (~4.9 us)
    tensor: 4x 512-col matmuls      (~2.9 us)
"""

import os
import sys

if "/opt/trn_rl_repo" not in sys.path:
    sys.path.insert(0, "/opt/trn_rl_repo")

import numpy as np

import concourse.bacc as bacc
import concourse.mybir as mybir
from concourse.tile import TileContext
from concourse.bass_utils import run_bass_kernel_spmd

F32 = mybir.dt.float32
BF16 = mybir.dt.bfloat16

T = 1000
C = 32
B = 8
H = 256
W = 256
HW = H * W

NCORES = 8
G = 4                 # spatial blocks packed into the 128 partitions
P = G * C             # 128
COLS = HW // G        # 16384 columns per spatial block
MM_N = 512            # max moving free-dim per matmul into one PSUM bank


def _cfg():
    return {
        "nt": int(os.environ.get("KCFG_NT", "512")),      # matmul chunk
        "nte": int(os.environ.get("KCFG_NTE", "1024")),   # PSUM tile / o-mult chunk
        "ntl": int(os.environ.get("KCFG_NTL", "2048")),   # DMA tile
        "new": int(os.environ.get("KCFG_NEW", "1024")),   # elementwise chunk
        "io": os.environ.get("KCFG_IO", "bf16"),          # bf16 | f32 (HBM I/O dtype)
        "mm": os.environ.get("KCFG_MM", "bf16"),          # bf16 | f32 (matmul dtype)
        "tt": os.environ.get("KCFG_TT", "gpsimd"),        # vector | gpsimd (g-mul)
        "defer": int(os.environ.get("KCFG_DEFER", "1")),  # iters to defer stores
        "ysrc": os.environ.get("KCFG_YSRC", "sp"),        # sp | act (y-load ring)
        "store": os.environ.get("KCFG_STORE", "pool"),    # pool | sp | act
        "ldbufs": int(os.environ.get("KCFG_LDBUFS", "6")),
        "wkbufs": int(os.environ.get("KCFG_WKBUFS", "4")),
        "psbufs": int(os.environ.get("KCFG_PSBUFS", "4")),
    }


_CACHE = {}


def _build():
    cfg = _cfg()
    key = tuple(sorted(cfg.items()))
    if key in _CACHE:
        return _CACHE[key]

    NT = cfg["nt"]
    NTE = cfg["nte"]
    NTL = cfg["ntl"]
    NEW = cfg["new"]
    assert NTL % NTE == 0 and NTE % NT == 0 and NT <= MM_N and NTL % NEW == 0
    widths = [NTL] * (COLS // NTL)
    assert sum(widths) == COLS

    nc = bacc.Bacc(
        "TRN2",
        target_bir_lowering=False,
        debug=False,
        enable_asserts=False,
        num_devices=NCORES,
    )

    MMDT = BF16 if cfg["mm"] == "bf16" else F32
    IODT = BF16 if cfg["io"] == "bf16" else F32
    xt_d = nc.dram_tensor("xt", [P, COLS], IODT, kind="ExternalInput")
    x0_d = nc.dram_tensor("x0", [P, COLS], IODT, kind="ExternalInput")
    mb_d = nc.dram_tensor("mb", [P, P], MMDT, kind="ExternalInput")
    sc_d = nc.dram_tensor("sc", [P, 4], F32, kind="ExternalInput")
    out_d = nc.dram_tensor("out", [P, COLS], IODT, kind="ExternalOutput")

    AF = mybir.ActivationFunctionType
    y_eng = nc.scalar if cfg["ysrc"] == "act" else nc.sync
    tt_eng = nc.vector if cfg["tt"] == "vector" else nc.gpsimd
    store_eng = {"pool": nc.gpsimd, "sp": nc.sync, "act": nc.scalar}[cfg["store"]]

    with TileContext(nc) as tc:
        with (
            tc.tile_pool(name="consts", bufs=1) as cpool,
            tc.tile_pool(name="work", bufs=cfg["wkbufs"]) as pool,
            tc.tile_pool(name="psum", bufs=cfg["psbufs"], space="PSUM") as psum,
        ):
            sc = cpool.tile([P, 4], F32)
            nc.scalar.dma_start(sc[:, :], sc_d[:, :])
            a_col = sc[:, 0:1]      # a
            k1_col = sc[:, 1:2]     # (1-a)/C
            ca_col = sc[:, 2:3]     # ca*a
            cb_col = sc[:, 3:4]     # ca*k1 + u
            mb = cpool.tile([P, P], MMDT)
            nc.scalar.dma_start(mb[:, :], mb_d[:, :])
            mb_mm = mb[:, :]

            pending = []

            def flush_store(po, poff, pw, eng):
                eng.dma_start(out_d[:, poff:poff + pw], po[:, :])

            off = 0
            for i, Wd in enumerate(widths):
                NE = Wd // NTE
                sl = slice(off, off + Wd)
                x = pool.tile([P, Wd], IODT, bufs=cfg["ldbufs"], tag="x",
                              padded_shape=[P, NTL], name=f"x_{i}")
                nc.sync.dma_start(x[:, :], xt_d[:, sl])
                y = pool.tile([P, Wd], IODT, bufs=cfg["ldbufs"], tag="y",
                              padded_shape=[P, NTL], name=f"y_{i}")
                y_eng.dma_start(y[:, :], x0_d[:, sl])
                o = pool.tile([P, Wd], IODT, bufs=cfg["ldbufs"], tag="o",
                              padded_shape=[P, NTL], name=f"o_{i}")

                # dn = (ca*a)*x + (ca*k1+u) == denom  (f32; sum_c p = 1
                # identity), p = a*x + k1, rden = 1/dn, g = x0 * rden.
                # NEW-wide sub-chunks keep the cross-engine pipeline fine-
                # grained so downstream stages start early.
                dn = pool.tile([P, Wd], F32, tag="dn", padded_shape=[P, NTL],
                               name=f"dn_{i}")
                p = pool.tile([P, Wd], F32, tag="p", padded_shape=[P, NTL],
                              name=f"p_{i}")
                rden = pool.tile([P, Wd], F32, tag="rden", padded_shape=[P, NTL],
                                 name=f"rden_{i}")
                g = pool.tile([P, Wd], MMDT, tag="g", padded_shape=[P, NTL],
                              name=f"g_{i}")
                for w in range(Wd // NEW):
                    ws = slice(w * NEW, (w + 1) * NEW)
                    nc.scalar.activation(dn[:, ws], x[:, ws], AF.Identity,
                                         bias=cb_col, scale=ca_col)
                    nc.vector.reciprocal_approx_fast(out=rden[:, ws], in_=dn[:, ws])
                    nc.scalar.activation(p[:, ws], x[:, ws], AF.Identity,
                                         bias=k1_col, scale=a_col)
                    tt_eng.tensor_tensor(g[:, ws], y[:, ws], rden[:, ws],
                                         mybir.AluOpType.mult)

                if len(pending) >= cfg["defer"]:
                    flush_store(*pending.pop(0))

                # r = kron(M, I4)^T @ g      (M symmetric); out = p * r
                for e in range(NE):
                    es = slice(e * NTE, (e + 1) * NTE)
                    r = psum.tile([P, NTE], F32, tag="r", name=f"r_{i}_{e}")
                    for s in range(NTE // NT):
                        ss = slice(s * NT, (s + 1) * NT)
                        gsl = slice(e * NTE + s * NT, e * NTE + (s + 1) * NT)
                        nc.tensor.matmul(r[:, ss], mb_mm, g[:, gsl],
                                         start=True, stop=True)
                    nc.vector.tensor_tensor(o[:, es], p[:, es], r[:, :],
                                            mybir.AluOpType.mult)

                pending.append((o, off, Wd, store_eng))
                off += Wd

            for args in pending:
                flush_store(*args)

    nc.compile()
    _CACHE[key] = nc
    return nc


def _host_prep(inputs):
    import ml_dtypes

    cfg = _cfg()
    iodt = ml_dtypes.bfloat16 if cfg["io"] == "bf16" else np.float32
    mmdt = ml_dtypes.bfloat16 if cfg["mm"] == "bf16" else np.float32
    xt = np.ascontiguousarray(np.asarray(inputs["xt"], dtype=np.float32).astype(iodt))
    x0 = np.ascontiguousarray(np.asarray(inputs["theta_x0"], dtype=np.float32).astype(iodt))
    t = np.asarray(inputs["t"]).astype(np.int64)
    al = np.asarray(inputs["alphas"], dtype=np.float32)
    cu = np.asarray(inputs["cumalphas"], dtype=np.float32)

    eyeC = np.eye(C, dtype=np.float64)
    eyeG = np.eye(G, dtype=np.float64)
    in_maps = []
    for b in range(B):
        tm = int(t[b]) - 1
        a = 0.0 if tm == 0 else float(al[tm])
        ca = 1.0 if tm == 0 else float(cu[tm - 1])
        u = (1.0 - ca) / C
        k1 = (1.0 - a) / C
        M = ca * eyeC + u
        mb = np.kron(M, eyeG).astype(mmdt)
        sc = np.empty((P, 4), dtype=np.float32)
        sc[:, 0] = a
        sc[:, 1] = k1
        sc[:, 2] = ca * a
        sc[:, 3] = ca * k1 + u
        in_maps.append(
            {
                "xt": xt[b].reshape(P, COLS),
                "x0": x0[b].reshape(P, COLS),
                "mb": mb,
                "sc": sc,
            }
        )
    return in_maps


def _run(inputs, trace=False, **kw):
    nc = _build()
    in_maps = _host_prep(inputs)
    res = run_bass_kernel_spmd(
        nc, in_maps, core_ids=list(range(NCORES)), trace=trace, **kw
    )
    out = np.stack(
        [r["out"].astype(np.float32).reshape(C, H, W) for r in res.results]
    )
    return out, res


def kernel(**inputs):
    out, _ = _run(inputs, trace=False)
    return out
